# revision 1
# baseline (speedup 1.0000x reference)
"""ChannelsSelectedConv2d on 8 Trainium2 NeuronCores.

Problem: for each output channel o (64 total), gather K=8 input channels
sel[o] from X [B=32, C=64, H=112, W=112], convolve with weight[o] [8,5,5]
(VALID), add bias[o] -> out [32, 64, 108, 108].

Strategy:
  - Shard the batch across 8 cores (4 images each, processed as 2 pairs).
  - The channel gather + grouped conv is recast as a dense 64x64
    channel-mixing matmul per kernel tap (dy, dx): on the host, scatter
    weight[o, j, dy, dx] into W64[tap, sel[o, j], o]. Then
      out[b, o, y, x] = sum_tap sum_c W64[tap, c, o] * X[b, c, y+dy, x+dx]
    which is exact for ANY sel (duplicates included).
  - On device: 2 images share one matmul (K = 2*64 channels on partitions,
    M = 2*64 output channels, block-diagonal lhsT), 25 PSUM-accumulated
    matmuls per output chunk of 4 rows (N = 4*108 = 432 columns).
  - X is cast to bf16 on the host (PE runs bf16 at full rate; PSUM
    accumulates fp32); output stays fp32.
"""

import numpy as np
import ml_dtypes

B, C, H, W = 32, 64, 112, 112
O, K, KS = 64, 8, 5
HO, WO = H - KS + 1, W - KS + 1  # 108, 108
N_CORES = 8
IMGS_PER_CORE = B // N_CORES  # 4
PAIRS = IMGS_PER_CORE // 2  # 2
ROWS_PER_CHUNK = 4
N_CHUNKS = HO // ROWS_PER_CHUNK  # 27
NTAPS = KS * KS  # 25

_built = None


def _build_nc():
    import concourse.bass as bass
    import concourse.tile as tile
    import concourse.mybir as mybir
    from concourse import bacc

    nc = bacc.Bacc(None)
    x = nc.dram_tensor("x", [PAIRS, 128, H, W], mybir.dt.bfloat16,
                       kind="ExternalInput")
    w = nc.dram_tensor("w", [128, NTAPS * 128], mybir.dt.bfloat16,
                       kind="ExternalInput")
    bt = nc.dram_tensor("b", [128, 1], mybir.dt.float32, kind="ExternalInput")
    y = nc.dram_tensor("y", [PAIRS, 128, HO * WO], mybir.dt.float32,
                       kind="ExternalOutput")

    with tile.TileContext(nc) as tc:
        with (
            tc.tile_pool(name="wp", bufs=1) as wp,
            tc.tile_pool(name="xp", bufs=2) as xp,
            tc.tile_pool(name="op", bufs=4) as op,
            tc.tile_pool(name="bp", bufs=1) as bp,
            tc.tile_pool(name="ps", bufs=8, space="PSUM") as ps,
        ):
            wt = wp.tile([128, NTAPS, 128], mybir.dt.bfloat16)
            nc.sync.dma_start(wt[:], w.rearrange("p (t m) -> p t m", t=NTAPS))
            bias = bp.tile([128, 1], mybir.dt.float32)
            nc.sync.dma_start(bias[:], bt[:])

            for pair in range(PAIRS):
                xt = xp.tile([128, H, W], mybir.dt.bfloat16)
                nc.sync.dma_start(xt[:], x[pair])
                for chunk in range(N_CHUNKS):
                    y0 = chunk * ROWS_PER_CHUNK
                    pt = ps.tile([128, ROWS_PER_CHUNK * WO], mybir.dt.float32)
                    t = 0
                    for dy in range(KS):
                        for dx in range(KS):
                            nc.tensor.matmul(
                                pt[:],
                                wt[:, t, :],
                                xt[:, y0 + dy : y0 + dy + ROWS_PER_CHUNK,
                                   dx : dx + WO],
                                start=(t == 0),
                                stop=(t == NTAPS - 1),
                            )
                            t += 1
                    ot = op.tile([128, ROWS_PER_CHUNK * WO], mybir.dt.float32)
                    nc.scalar.add(ot[:], pt[:], bias[:])
                    nc.sync.dma_start(
                        y[pair][:, y0 * WO : (y0 + ROWS_PER_CHUNK) * WO], ot[:]
                    )
    nc.finalize()
    return nc


def _prep_inputs(X, weight, bias, sel):
    """Host-side prep: weight scatter, layout, bf16 cast, batch shard."""
    # Dense per-tap channel-mix matrix: W64[tap, c, o]
    w64 = np.zeros((NTAPS, C, O), dtype=np.float32)
    wflat = weight.reshape(O, K, NTAPS).astype(np.float32)
    for o in range(O):
        for j in range(K):
            w64[:, int(sel[o, j]), o] += wflat[o, j]
    # Block-diagonal 2-image lhsT: [tap, 128, 128]
    lhsT = np.zeros((NTAPS, 128, 128), dtype=np.float32)
    lhsT[:, :C, :O] = w64
    lhsT[:, C:, O:] = w64
    # SBUF layout [partition, tap*128], bf16
    w_host = np.ascontiguousarray(
        lhsT.transpose(1, 0, 2).reshape(128, NTAPS * 128)
    ).astype(ml_dtypes.bfloat16)

    b_host = np.tile(np.asarray(bias, dtype=np.float32), 2).reshape(128, 1)

    xb = np.asarray(X, dtype=np.float32).astype(ml_dtypes.bfloat16)
    # [B, C, H, W] -> per-core [PAIRS, 2*C, H, W]
    xcores = xb.reshape(N_CORES, PAIRS, 2 * C, H, W)

    in_maps = [
        {"x": np.ascontiguousarray(xcores[i]), "w": w_host, "b": b_host}
        for i in range(N_CORES)
    ]
    return in_maps


def _postprocess(results):
    outs = []
    for r in results:
        yc = r["y"].reshape(PAIRS * 2, O, HO, WO)
        outs.append(yc)
    return np.concatenate(outs, axis=0).astype(np.float32)


def kernel(X, weight, bias, sel):
    global _built
    from concourse.bass_utils import run_bass_kernel_spmd

    assert X.shape == (B, C, H, W), X.shape
    assert weight.shape == (O, K, KS, KS), weight.shape
    assert sel.shape == (O, K), sel.shape

    if _built is None:
        _built = _build_nc()

    in_maps = _prep_inputs(X, weight, bias, sel)
    res = run_bass_kernel_spmd(
        _built, in_maps, core_ids=list(range(N_CORES)), trace=False
    )
    return _postprocess(res.results)


# revision 2
# speedup vs baseline: 1.0233x; 1.0233x over previous
"""ChannelsSelectedConv2d on 8 Trainium2 NeuronCores.

Problem: for each output channel o (64 total), gather K=8 input channels
sel[o] from X [B=32, C=64, H=112, W=112], convolve with weight[o] [8,5,5]
(VALID), add bias[o] -> out [32, 64, 108, 108].

Strategy:
  - Shard the batch across 8 cores (4 images each, processed as 2 pairs).
  - The channel gather + grouped conv is recast as a dense 64x64
    channel-mixing matmul per kernel tap (dy, dx): on the host, scatter
    weight[o, j, dy, dx] into W64[tap, sel[o, j], o]. Then
      out[b, o, y, x] = sum_tap sum_c W64[tap, c, o] * X[b, c, y+dy, x+dx]
    which is exact for ANY sel (duplicates included).
  - On device: 2 images share one matmul (K = 2*64 channels on partitions,
    M = 2*64 output channels, block-diagonal lhsT), 25 PSUM-accumulated
    matmuls per output chunk of 4 rows (N = 4*108 = 432 columns).
  - X is cast to bf16 on the host (PE runs bf16 at full rate; PSUM
    accumulates fp32); output stays fp32.
"""

import numpy as np
import ml_dtypes

B, C, H, W = 32, 64, 112, 112
O, K, KS = 64, 8, 5
HO, WO = H - KS + 1, W - KS + 1  # 108, 108
N_CORES = 8
IMGS_PER_CORE = B // N_CORES  # 4
PAIRS = IMGS_PER_CORE // 2  # 2
ROWS_PER_CHUNK = 4
N_CHUNKS = HO // ROWS_PER_CHUNK  # 27
NTAPS = KS * KS  # 25

_built = None


def _build_nc():
    import concourse.bass as bass
    import concourse.tile as tile
    import concourse.mybir as mybir
    from concourse import bacc

    nc = bacc.Bacc(None)
    x = nc.dram_tensor("x", [PAIRS, 128, H, W], mybir.dt.bfloat16,
                       kind="ExternalInput")
    w = nc.dram_tensor("w", [128, NTAPS * 128], mybir.dt.bfloat16,
                       kind="ExternalInput")
    bt = nc.dram_tensor("b", [128, 1], mybir.dt.float32, kind="ExternalInput")
    y = nc.dram_tensor("y", [PAIRS, 128, HO * WO], mybir.dt.float32,
                       kind="ExternalOutput")

    with tile.TileContext(nc) as tc:
        with (
            tc.tile_pool(name="wp", bufs=1) as wp,
            tc.tile_pool(name="xp", bufs=2) as xp,
            tc.tile_pool(name="op", bufs=4) as op,
            tc.tile_pool(name="bp", bufs=1) as bp,
            tc.tile_pool(name="ps", bufs=8, space="PSUM") as ps,
        ):
            wt = wp.tile([128, NTAPS, 128], mybir.dt.bfloat16)
            nc.sync.dma_start(wt[:], w.rearrange("p (t m) -> p t m", t=NTAPS))
            bias = bp.tile([128, 1], mybir.dt.float32)
            nc.sync.dma_start(bias[:], bt[:])

            BAND = 16  # X rows per input DMA; lets compute start early
            for pair in range(PAIRS):
                xt = xp.tile([128, H, W], mybir.dt.bfloat16)
                for b0 in range(0, H, BAND):
                    nc.sync.dma_start(
                        xt[:, b0 : b0 + BAND, :], x[pair][:, b0 : b0 + BAND, :]
                    )
                for chunk in range(N_CHUNKS):
                    y0 = chunk * ROWS_PER_CHUNK
                    pt = ps.tile([128, ROWS_PER_CHUNK * WO], mybir.dt.float32)
                    t = 0
                    for dy in range(KS):
                        for dx in range(KS):
                            nc.tensor.matmul(
                                pt[:],
                                wt[:, t, :],
                                xt[:, y0 + dy : y0 + dy + ROWS_PER_CHUNK,
                                   dx : dx + WO],
                                start=(t == 0),
                                stop=(t == NTAPS - 1),
                            )
                            t += 1
                    ot = op.tile([128, ROWS_PER_CHUNK * WO], mybir.dt.float32)
                    nc.vector.tensor_scalar_add(ot[:], pt[:], bias[:])
                    nc.sync.dma_start(
                        y[pair][:, y0 * WO : (y0 + ROWS_PER_CHUNK) * WO], ot[:]
                    )
    nc.finalize()
    return nc


def _prep_inputs(X, weight, bias, sel):
    """Host-side prep: weight scatter, layout, bf16 cast, batch shard."""
    # Dense per-tap channel-mix matrix: W64[tap, c, o]
    w64 = np.zeros((NTAPS, C, O), dtype=np.float32)
    wflat = weight.reshape(O, K, NTAPS).astype(np.float32)
    for o in range(O):
        for j in range(K):
            w64[:, int(sel[o, j]), o] += wflat[o, j]
    # Block-diagonal 2-image lhsT: [tap, 128, 128]
    lhsT = np.zeros((NTAPS, 128, 128), dtype=np.float32)
    lhsT[:, :C, :O] = w64
    lhsT[:, C:, O:] = w64
    # SBUF layout [partition, tap*128], bf16
    w_host = np.ascontiguousarray(
        lhsT.transpose(1, 0, 2).reshape(128, NTAPS * 128)
    ).astype(ml_dtypes.bfloat16)

    b_host = np.tile(np.asarray(bias, dtype=np.float32), 2).reshape(128, 1)

    xb = np.asarray(X, dtype=np.float32).astype(ml_dtypes.bfloat16)
    # [B, C, H, W] -> per-core [PAIRS, 2*C, H, W]
    xcores = xb.reshape(N_CORES, PAIRS, 2 * C, H, W)

    in_maps = [
        {"x": np.ascontiguousarray(xcores[i]), "w": w_host, "b": b_host}
        for i in range(N_CORES)
    ]
    return in_maps


def _postprocess(results):
    outs = []
    for r in results:
        yc = r["y"].reshape(PAIRS * 2, O, HO, WO)
        outs.append(yc)
    return np.concatenate(outs, axis=0).astype(np.float32)


def kernel(X, weight, bias, sel):
    global _built
    from concourse.bass_utils import run_bass_kernel_spmd

    assert X.shape == (B, C, H, W), X.shape
    assert weight.shape == (O, K, KS, KS), weight.shape
    assert sel.shape == (O, K), sel.shape

    if _built is None:
        _built = _build_nc()

    in_maps = _prep_inputs(X, weight, bias, sel)
    res = run_bass_kernel_spmd(
        _built, in_maps, core_ids=list(range(N_CORES)), trace=False
    )
    return _postprocess(res.results)


# revision 4
# speedup vs baseline: 1.0241x; 1.0008x over previous
"""ChannelsSelectedConv2d on 8 Trainium2 NeuronCores.

Problem: for each output channel o (64 total), gather K=8 input channels
sel[o] from X [B=32, C=64, H=112, W=112], convolve with weight[o] [8,5,5]
(VALID), add bias[o] -> out [32, 64, 108, 108].

Strategy:
  - Shard the batch across 8 cores (4 images each, processed as 2 pairs).
  - The channel gather + grouped conv is recast as a dense 64x64
    channel-mixing matmul per kernel tap (dy, dx): on the host, scatter
    weight[o, j, dy, dx] into W64[tap, sel[o, j], o]. Then
      out[b, o, y, x] = sum_tap sum_c W64[tap, c, o] * X[b, c, y+dy, x+dx]
    which is exact for ANY sel (duplicates included).
  - On device: 2 images share one matmul (K = 2*64 channels on partitions,
    M = 2*64 output channels, block-diagonal lhsT), 25 PSUM-accumulated
    matmuls per output chunk of 4 rows (N = 4*108 = 432 columns).
  - X is cast to bf16 on the host (PE runs bf16 at full rate; PSUM
    accumulates fp32); output stays fp32.
"""

import numpy as np
import ml_dtypes

B, C, H, W = 32, 64, 112, 112
O, K, KS = 64, 8, 5
HO, WO = H - KS + 1, W - KS + 1  # 108, 108
N_CORES = 8
IMGS_PER_CORE = B // N_CORES  # 4
PAIRS = IMGS_PER_CORE // 2  # 2
ROWS_PER_CHUNK = 4
N_CHUNKS = HO // ROWS_PER_CHUNK  # 27
NTAPS = KS * KS  # 25

_built = None


def _build_nc():
    import concourse.bass as bass
    import concourse.tile as tile
    import concourse.mybir as mybir
    from concourse import bacc

    nc = bacc.Bacc(None)
    x = nc.dram_tensor("x", [PAIRS, 128, H, W], mybir.dt.bfloat16,
                       kind="ExternalInput")
    w = nc.dram_tensor("w", [128, NTAPS * 128], mybir.dt.bfloat16,
                       kind="ExternalInput")
    bt = nc.dram_tensor("b", [128, 1], mybir.dt.float32, kind="ExternalInput")
    y = nc.dram_tensor("y", [PAIRS, 128, HO * WO], mybir.dt.float32,
                       kind="ExternalOutput")

    with tile.TileContext(nc) as tc:
        with (
            tc.tile_pool(name="wp", bufs=1) as wp,
            tc.tile_pool(name="xp", bufs=2) as xp,
            tc.tile_pool(name="op", bufs=4) as op,
            tc.tile_pool(name="bp", bufs=1) as bp,
            tc.tile_pool(name="ps", bufs=8, space="PSUM") as ps,
        ):
            w3 = w.rearrange("p (t m) -> p t m", t=NTAPS)
            wt = wp.tile([128, NTAPS, 128], mybir.dt.bfloat16)
            # tap-0 weights first so the very first matmul is unblocked early
            nc.sync.dma_start(wt[:, 0:1, :], w3[:, 0:1, :])

            xtiles = []
            BAND = 16  # X rows per input DMA; lets compute start early
            for pair in range(PAIRS):
                xt = xp.tile([128, H, W], mybir.dt.bfloat16, tag="xt")
                xtiles.append(xt)
                # first 8 rows = exactly what output chunk 0 needs
                nc.sync.dma_start(xt[:, 0:8, :], x[pair][:, 0:8, :])
                if pair == 0:
                    nc.sync.dma_start(wt[:, 1:, :], w3[:, 1:, :])
                    bias = bp.tile([128, 1], mybir.dt.float32)
                    nc.sync.dma_start(bias[:], bt[:])
                for b0 in range(8, H, BAND):
                    b1 = min(b0 + BAND, H)
                    nc.sync.dma_start(
                        xt[:, b0:b1, :], x[pair][:, b0:b1, :]
                    )

            for pair in range(PAIRS):
                xt = xtiles[pair]
                for chunk in range(N_CHUNKS):
                    y0 = chunk * ROWS_PER_CHUNK
                    pt = ps.tile([128, ROWS_PER_CHUNK * WO], mybir.dt.float32)
                    t = 0
                    for dy in range(KS):
                        for dx in range(KS):
                            nc.tensor.matmul(
                                pt[:],
                                wt[:, t, :],
                                xt[:, y0 + dy : y0 + dy + ROWS_PER_CHUNK,
                                   dx : dx + WO],
                                start=(t == 0),
                                stop=(t == NTAPS - 1),
                            )
                            t += 1
                    ot = op.tile([128, ROWS_PER_CHUNK * WO], mybir.dt.float32)
                    nc.vector.tensor_scalar_add(ot[:], pt[:], bias[:])
                    nc.sync.dma_start(
                        y[pair][:, y0 * WO : (y0 + ROWS_PER_CHUNK) * WO], ot[:]
                    )
    nc.finalize()
    return nc


def _prep_inputs(X, weight, bias, sel):
    """Host-side prep: weight scatter, layout, bf16 cast, batch shard."""
    # Dense per-tap channel-mix matrix: W64[tap, c, o]
    w64 = np.zeros((NTAPS, C, O), dtype=np.float32)
    wflat = weight.reshape(O, K, NTAPS).astype(np.float32)
    for o in range(O):
        for j in range(K):
            w64[:, int(sel[o, j]), o] += wflat[o, j]
    # Block-diagonal 2-image lhsT: [tap, 128, 128]
    lhsT = np.zeros((NTAPS, 128, 128), dtype=np.float32)
    lhsT[:, :C, :O] = w64
    lhsT[:, C:, O:] = w64
    # SBUF layout [partition, tap*128], bf16
    w_host = np.ascontiguousarray(
        lhsT.transpose(1, 0, 2).reshape(128, NTAPS * 128)
    ).astype(ml_dtypes.bfloat16)

    b_host = np.tile(np.asarray(bias, dtype=np.float32), 2).reshape(128, 1)

    xb = np.asarray(X, dtype=np.float32).astype(ml_dtypes.bfloat16)
    # [B, C, H, W] -> per-core [PAIRS, 2*C, H, W]
    xcores = xb.reshape(N_CORES, PAIRS, 2 * C, H, W)

    in_maps = [
        {"x": np.ascontiguousarray(xcores[i]), "w": w_host, "b": b_host}
        for i in range(N_CORES)
    ]
    return in_maps


def _postprocess(results):
    outs = []
    for r in results:
        yc = r["y"].reshape(PAIRS * 2, O, HO, WO)
        outs.append(yc)
    return np.concatenate(outs, axis=0).astype(np.float32)


def kernel(X, weight, bias, sel):
    global _built
    from concourse.bass_utils import run_bass_kernel_spmd

    assert X.shape == (B, C, H, W), X.shape
    assert weight.shape == (O, K, KS, KS), weight.shape
    assert sel.shape == (O, K), sel.shape

    if _built is None:
        _built = _build_nc()

    in_maps = _prep_inputs(X, weight, bias, sel)
    res = run_bass_kernel_spmd(
        _built, in_maps, core_ids=list(range(N_CORES)), trace=False
    )
    return _postprocess(res.results)


# revision 7
# speedup vs baseline: 1.0428x; 1.0182x over previous
"""ChannelsSelectedConv2d on 8 Trainium2 NeuronCores.

Problem: for each output channel o (64 total), gather K=8 input channels
sel[o] from X [B=32, C=64, H=112, W=112], convolve with weight[o] [8,5,5]
(VALID), add bias[o] -> out [32, 64, 108, 108].

Strategy:
  - Shard the batch across 8 cores (4 images each, processed as 2 pairs).
  - The channel gather + grouped conv is recast as a dense 64x64
    channel-mixing matmul per kernel tap (dy, dx): on the host, scatter
    weight[o, j, dy, dx] into W64[tap, sel[o, j], o]. Then
      out[b, o, y, x] = sum_tap sum_c W64[tap, c, o] * X[b, c, y+dy, x+dx]
    which is exact for ANY sel (duplicates included).
  - On device: 2 images share one matmul (K = 2*64 channels on partitions,
    M = 2*64 output channels, block-diagonal lhsT), 25 PSUM-accumulated
    matmuls per output chunk of 4 rows (N = 4*108 = 432 columns).
  - X is cast to bf16 on the host (PE runs bf16 at full rate; PSUM
    accumulates fp32); output stays fp32.
"""

import numpy as np
import ml_dtypes

B, C, H, W = 32, 64, 112, 112
O, K, KS = 64, 8, 5
HO, WO = H - KS + 1, W - KS + 1  # 108, 108
N_CORES = 8
IMGS_PER_CORE = B // N_CORES  # 4
PAIRS = IMGS_PER_CORE // 2  # 2
ROWS_PER_CHUNK = 4
N_CHUNKS = HO // ROWS_PER_CHUNK  # 27
NTAPS = KS * KS  # 25

_built = None


def _build_nc():
    import concourse.bass as bass
    import concourse.tile as tile
    import concourse.mybir as mybir
    from concourse import bacc

    nc = bacc.Bacc(None)
    x = nc.dram_tensor("x", [PAIRS, 128, H, W], mybir.dt.bfloat16,
                       kind="ExternalInput")
    w = nc.dram_tensor("w", [128, NTAPS * 128], mybir.dt.bfloat16,
                       kind="ExternalInput")
    bt = nc.dram_tensor("b", [128, 1], mybir.dt.float32, kind="ExternalInput")
    y = nc.dram_tensor("y", [PAIRS, 128, HO * WO], mybir.dt.float32,
                       kind="ExternalOutput")

    with tile.TileContext(nc) as tc:
        with (
            tc.tile_pool(name="wp", bufs=1) as wp,
            tc.tile_pool(name="xp", bufs=2) as xp,
            tc.tile_pool(name="op", bufs=4) as op,
            tc.tile_pool(name="bp", bufs=1) as bp,
            tc.tile_pool(name="ps", bufs=8, space="PSUM") as ps,
        ):
            # Warm the PE's HAM clock gate during the input-DMA fill so the
            # real matmuls run at 2.4 GHz from the start (~3.4us of PE
            # activity flips K from 4/8 to 8/8).
            warm = wp.tile([128, 64], mybir.dt.bfloat16, tag="warm")
            nc.vector.memset(warm[:], 0.0)
            wpsum = ps.tile([128, 64], mybir.dt.float32, tag="pt")
            for _ in range(70):
                nc.tensor.matmul(wpsum[:64, :], warm[:], warm[:],
                                 start=True, stop=True)

            w3 = w.rearrange("p (t m) -> p t m", t=NTAPS)
            wt = wp.tile([128, NTAPS, 128], mybir.dt.bfloat16)

            xtiles = []
            BAND = 16  # X rows per input DMA; lets compute start early
            for pair in range(PAIRS):
                xt = xp.tile([128, H, W], mybir.dt.bfloat16, tag="xt")
                xtiles.append(xt)
                # first 8 rows = exactly what output chunk 0 needs
                nc.sync.dma_start(xt[:, 0:8, :], x[pair][:, 0:8, :])
                if pair == 0:
                    # weights split so early taps unblock before the full load
                    nc.sync.dma_start(wt[:, 0:1, :], w3[:, 0:1, :])
                    for ts in ((1, 7), (7, 13), (13, 19), (19, NTAPS)):
                        nc.sync.dma_start(wt[:, ts[0]:ts[1], :], w3[:, ts[0]:ts[1], :])
                    bias = bp.tile([128, 1], mybir.dt.float32)
                    nc.sync.dma_start(bias[:], bt[:])
                for b0 in range(8, H, BAND):
                    b1 = min(b0 + BAND, H)
                    nc.sync.dma_start(
                        xt[:, b0:b1, :], x[pair][:, b0:b1, :]
                    )

            for pair in range(PAIRS):
                xt = xtiles[pair]
                for chunk in range(N_CHUNKS):
                    y0 = chunk * ROWS_PER_CHUNK
                    pt = ps.tile([128, ROWS_PER_CHUNK * WO], mybir.dt.float32, tag="pt")
                    t = 0
                    for dy in range(KS):
                        for dx in range(KS):
                            nc.tensor.matmul(
                                pt[:],
                                wt[:, t, :],
                                xt[:, y0 + dy : y0 + dy + ROWS_PER_CHUNK,
                                   dx : dx + WO],
                                start=(t == 0),
                                stop=(t == NTAPS - 1),
                            )
                            t += 1
                    ot = op.tile([128, ROWS_PER_CHUNK * WO], mybir.dt.float32)
                    nc.vector.tensor_scalar_add(ot[:], pt[:], bias[:])
                    nc.sync.dma_start(
                        y[pair][:, y0 * WO : (y0 + ROWS_PER_CHUNK) * WO], ot[:]
                    )
    nc.finalize()
    return nc


def _prep_inputs(X, weight, bias, sel):
    """Host-side prep: weight scatter, layout, bf16 cast, batch shard."""
    # Dense per-tap channel-mix matrix: W64[tap, c, o]
    w64 = np.zeros((NTAPS, C, O), dtype=np.float32)
    wflat = weight.reshape(O, K, NTAPS).astype(np.float32)
    for o in range(O):
        for j in range(K):
            w64[:, int(sel[o, j]), o] += wflat[o, j]
    # Block-diagonal 2-image lhsT: [tap, 128, 128]
    lhsT = np.zeros((NTAPS, 128, 128), dtype=np.float32)
    lhsT[:, :C, :O] = w64
    lhsT[:, C:, O:] = w64
    # SBUF layout [partition, tap*128], bf16
    w_host = np.ascontiguousarray(
        lhsT.transpose(1, 0, 2).reshape(128, NTAPS * 128)
    ).astype(ml_dtypes.bfloat16)

    b_host = np.tile(np.asarray(bias, dtype=np.float32), 2).reshape(128, 1)

    xb = np.asarray(X, dtype=np.float32).astype(ml_dtypes.bfloat16)
    # [B, C, H, W] -> per-core [PAIRS, 2*C, H, W]
    xcores = xb.reshape(N_CORES, PAIRS, 2 * C, H, W)

    in_maps = [
        {"x": np.ascontiguousarray(xcores[i]), "w": w_host, "b": b_host}
        for i in range(N_CORES)
    ]
    return in_maps


def _postprocess(results):
    outs = []
    for r in results:
        yc = r["y"].reshape(PAIRS * 2, O, HO, WO)
        outs.append(yc)
    return np.concatenate(outs, axis=0).astype(np.float32)


def kernel(X, weight, bias, sel):
    global _built
    from concourse.bass_utils import run_bass_kernel_spmd

    assert X.shape == (B, C, H, W), X.shape
    assert weight.shape == (O, K, KS, KS), weight.shape
    assert sel.shape == (O, K), sel.shape

    if _built is None:
        _built = _build_nc()

    in_maps = _prep_inputs(X, weight, bias, sel)
    res = run_bass_kernel_spmd(
        _built, in_maps, core_ids=list(range(N_CORES)), trace=False
    )
    return _postprocess(res.results)


# revision 8
# speedup vs baseline: 1.5960x; 1.5306x over previous
"""ChannelsSelectedConv2d on 8 Trainium2 NeuronCores.

Problem: for each output channel o (64 total), gather K=8 input channels
sel[o] from X [B=32, C=64, H=112, W=112], convolve with weight[o] [8,5,5]
(VALID), add bias[o] -> out [32, 64, 108, 108].

Strategy:
  - Shard the batch across 8 cores (4 images each, processed as 2 pairs).
  - The channel gather + grouped conv is recast as a dense 64x64
    channel-mixing matmul per kernel tap (dy, dx): on the host, scatter
    weight[o, j, dy, dx] into W64[tap, sel[o, j], o]. Then
      out[b, o, y, x] = sum_tap sum_c W64[tap, c, o] * X[b, c, y+dy, x+dx]
    which is exact for ANY sel (duplicates included).
  - On device: 2 images share one matmul (K = 2*64 channels on partitions,
    M = 2*64 output channels, block-diagonal lhsT), 25 PSUM-accumulated
    matmuls per output chunk of 4 rows (N = 4*108 = 432 columns).
  - X is cast to bf16 on the host (PE runs bf16 at full rate; PSUM
    accumulates fp32); output stays fp32.
"""

import numpy as np
import ml_dtypes

B, C, H, W = 32, 64, 112, 112
O, K, KS = 64, 8, 5
HO, WO = H - KS + 1, W - KS + 1  # 108, 108
N_CORES = 8
IMGS_PER_CORE = B // N_CORES  # 4
PAIRS = IMGS_PER_CORE // 2  # 2
ROWS_PER_CHUNK = 4
N_CHUNKS = HO // ROWS_PER_CHUNK  # 27
NTAPS = KS * KS  # 25

_built = None


def _build_nc():
    import concourse.bass as bass
    import concourse.tile as tile
    import concourse.mybir as mybir
    from concourse import bacc

    nc = bacc.Bacc(None)
    x = nc.dram_tensor("x", [PAIRS, 128, H, W], mybir.dt.bfloat16,
                       kind="ExternalInput")
    w = nc.dram_tensor("w", [128, NTAPS * 128], mybir.dt.bfloat16,
                       kind="ExternalInput")
    bt = nc.dram_tensor("b", [128, 1], mybir.dt.float32, kind="ExternalInput")
    y = nc.dram_tensor("y", [PAIRS, 128, HO * WO], mybir.dt.float32,
                       kind="ExternalOutput")

    with tile.TileContext(nc) as tc:
        with (
            tc.tile_pool(name="wp", bufs=1) as wp,
            tc.tile_pool(name="xp", bufs=2) as xp,
            tc.tile_pool(name="op", bufs=4) as op,
            tc.tile_pool(name="bp", bufs=1) as bp,
            tc.tile_pool(name="ps", bufs=8, space="PSUM") as ps,
        ):
            # Warm the PE's HAM clock gate during the input-DMA fill so the
            # real matmuls run at 2.4 GHz from the start (~3.4us of PE
            # activity flips K from 4/8 to 8/8).
            warm = wp.tile([128, 64], mybir.dt.bfloat16, tag="warm")
            nc.vector.memset(warm[:], 0.0)
            wpsum = ps.tile([128, 64], mybir.dt.float32, tag="pt")
            for _ in range(70):
                nc.tensor.matmul(wpsum[:64, :], warm[:], warm[:],
                                 start=True, stop=True)

            w3 = w.rearrange("p (t m) -> p t m", t=NTAPS)
            wt = wp.tile([128, NTAPS, 128], mybir.dt.bfloat16)

            xtiles = []
            BAND = 16  # X rows per input DMA; lets compute start early
            for pair in range(PAIRS):
                xt = xp.tile([128, H, W], mybir.dt.bfloat16, tag="xt")
                xtiles.append(xt)
                # first 8 rows = exactly what output chunk 0 needs
                nc.sync.dma_start(xt[:, 0:8, :], x[pair][:, 0:8, :])
                if pair == 0:
                    # weights split so early taps unblock before the full load
                    nc.sync.dma_start(wt[:, 0:1, :], w3[:, 0:1, :])
                    for ts in ((1, 7), (7, 13), (13, 19), (19, NTAPS)):
                        nc.sync.dma_start(wt[:, ts[0]:ts[1], :], w3[:, ts[0]:ts[1], :])
                    bias = bp.tile([128, 1], mybir.dt.float32)
                    nc.sync.dma_start(bias[:], bt[:])
                for b0 in range(8, H, BAND):
                    b1 = min(b0 + BAND, H)
                    nc.sync.dma_start(
                        xt[:, b0:b1, :], x[pair][:, b0:b1, :]
                    )

            for pair in range(PAIRS):
                xt = xtiles[pair]
                for chunk in range(N_CHUNKS):
                    y0 = chunk * ROWS_PER_CHUNK
                    pt = ps.tile([128, ROWS_PER_CHUNK * WO], mybir.dt.float32, tag="pt")
                    t = 0
                    for dy in range(KS):
                        for dx in range(KS):
                            nc.tensor.matmul(
                                pt[:],
                                wt[:, t, :],
                                xt[:, y0 + dy : y0 + dy + ROWS_PER_CHUNK,
                                   dx : dx + WO],
                                start=(t == 0),
                                stop=(t == NTAPS - 1),
                            )
                            t += 1
                    ot = op.tile([128, ROWS_PER_CHUNK * WO], mybir.dt.float32)
                    nc.vector.tensor_scalar_add(ot[:], pt[:], bias[:])
                    nc.sync.dma_start(
                        y[pair][:, y0 * WO : (y0 + ROWS_PER_CHUNK) * WO], ot[:]
                    )
    nc.finalize()
    return nc


def _prep_inputs(X, weight, bias, sel):
    """Host-side prep: weight scatter, layout, bf16 cast, batch shard."""
    weight = np.asarray(weight)
    sel = np.asarray(sel)
    # Dense per-tap channel-mix matrix: W64[tap, c, o]
    w64 = np.zeros((NTAPS, C, O), dtype=np.float32)
    wflat = weight.reshape(O, K, NTAPS).astype(np.float32)
    for o in range(O):
        for j in range(K):
            w64[:, int(sel[o, j]), o] += wflat[o, j]
    # Block-diagonal 2-image lhsT: [tap, 128, 128]
    lhsT = np.zeros((NTAPS, 128, 128), dtype=np.float32)
    lhsT[:, :C, :O] = w64
    lhsT[:, C:, O:] = w64
    # SBUF layout [partition, tap*128], bf16
    w_host = np.ascontiguousarray(
        lhsT.transpose(1, 0, 2).reshape(128, NTAPS * 128)
    ).astype(ml_dtypes.bfloat16)

    b_host = np.tile(np.asarray(bias, dtype=np.float32), 2).reshape(128, 1)

    xb = np.asarray(X, dtype=np.float32).astype(ml_dtypes.bfloat16)
    # [B, C, H, W] -> per-core [PAIRS, 2*C, H, W]
    xcores = xb.reshape(N_CORES, PAIRS, 2 * C, H, W)

    in_maps = [
        {"x": np.ascontiguousarray(xcores[i]), "w": w_host, "b": b_host}
        for i in range(N_CORES)
    ]
    return in_maps


def _postprocess(results):
    outs = []
    for r in results:
        yc = r["y"].reshape(PAIRS * 2, O, HO, WO)
        outs.append(yc)
    return np.concatenate(outs, axis=0).astype(np.float32)


def kernel(X, weight, bias, sel):
    global _built
    from concourse.bass_utils import run_bass_kernel_spmd

    assert X.shape == (B, C, H, W), X.shape
    assert weight.shape == (O, K, KS, KS), weight.shape
    assert sel.shape == (O, K), sel.shape

    if _built is None:
        _built = _build_nc()

    in_maps = _prep_inputs(X, weight, bias, sel)
    res = run_bass_kernel_spmd(
        _built, in_maps, core_ids=list(range(N_CORES)), trace=False
    )
    return _postprocess(res.results)
